# revision 1
# baseline (speedup 1.0000x reference)
"""Depth-masked 3-branch 3x3 conv (Conv2.5D) on 8 TRN2 NeuronCores.

Data-parallel over N=8 images (1 image/core). Per core:
  - masks encoded as r in {0..4} via nearest-integer binning of
    t = (d_col - center)/grid  (branch b active iff r == 3-b)
  - r computed compactly (128h x 9*128w), reshaped to a shuffle-source
    layout, broadcast across partitions with DVE stream_shuffle
  - masked inputs built with one fused scalar_tensor_tensor per
    (branch, tap-pair):  xm = (r_rep == 3-b) * x_shift   [bf16]
  - 15 K=128 bf16 matmuls per 512-px chunk accumulate all taps/branches
    into one PSUM tile (weights host-packed into pair-stacked lhsT)
"""

import sys

sys.path.insert(0, "/opt/trn_rl_repo")

import numpy as np
import ml_dtypes

import concourse.bass as bass
import concourse.mybir as mybir
from concourse.bass_utils import run_bass_kernel_spmd
from concourse import tile
from concourse.vector_clock import VectorClock, ScopedClock

F32 = mybir.dt.float32
BF16 = mybir.dt.bfloat16
AF = mybir.ActivationFunctionType
ALU = mybir.AluOpType

N_IMG, C, O, H, W = 8, 64, 64, 128, 128
L = H * W
CHUNK = 1024
NCHUNK = L // CHUNK
BASE = 256  # pad on each side of the x tiles (reads span +-130)
# tap k = 3*(dh+1)+(dw+1); flat pixel offset dh*W+dw
OFF = [(k // 3 - 1) * W + (k % 3 - 1) for k in range(9)]
# pairs sharing one physical x tile: (k1,k2) with off(k2)-off(k1)==delta
PATTERNS = [(0, 1), (3, 4), (6, 7), (2, 5), (8, 8)]  # delta: 1,1,1,128,(dup)


def _patched_drain_and_barrier(self, tick_clock, wait_clock):
    # stock version puts every live sem wait on one drain -> walrus
    # "Too many sync wait commands"; emit one single-wait NOP per sem.
    ticks = list(tick_clock.global_clock)
    n = len(ticks)
    for i, t in enumerate(ticks):
        if t > 0:
            vec = [0] * n
            vec[i] = t
            nop = self.nc.sync.nop()
            wait_clock.add_sem_waits(nop.ins, ScopedClock({None: VectorClock(vec)}))
    self.nc.sync.drain()
    self.nc.all_engine_barrier()
    popped = self.nc._tile_sem_poison_stack.pop()
    assert popped is self._sem_poison
    self.nc.clear_and_free_semaphores(list(self.sems.allocated().values()))
    self.nc.all_engine_barrier()


tile.TileContext._drain_and_barrier = _patched_drain_and_barrier


def _split_excess_waits(nc, noop_cls, max_waits=1):
    # this walrus build rejects >1 sync-wait on several instruction
    # structs; hoist extras onto same-engine NoOps placed just before.
    for fn in nc.m.functions:
        for blk in fn.blocks:
            idx = 0
            while idx < len(blk.instructions):
                inst = blk.instructions[idx]
                si = inst.sync_info
                if si is not None and len(si.on_wait) > max_waits:
                    waits = list(si.on_wait)
                    si.on_wait = waits[-max_waits:]
                    pos = idx
                    for w in waits[:-max_waits]:
                        nop = noop_cls(
                            name=nc.get_next_instruction_name(), ins=[], outs=[]
                        )
                        nop.engine = inst.engine
                        nop.sync_info = mybir.SyncInfo(on_wait=[w], on_update=[])
                        nc.register_instruction(nop)
                        blk.instructions.insert(pos, nop)
                        pos += 1
                        idx += 1
                idx += 1


def _build_graph():
    nc = bass.Bass()
    x_d = nc.declare_dram_parameter("x", [C, L], F32, isOutput=False)
    dep_d = nc.declare_dram_parameter("depth", [H, W], F32, isOutput=False)
    rfx_d = nc.declare_dram_parameter("rfx", [128, 1], F32, isOutput=False)
    wp_d = nc.declare_dram_parameter("wp", [128, 15 * 64], BF16, isOutput=False)
    out_d = nc.declare_dram_parameter("out", [O, L], F32, isOutput=True)

    XW = BASE + L + BASE
    with tile.TileContext(nc) as tc:
        with (
            tc.tile_pool(name="big", bufs=1) as big,
            tc.tile_pool(name="mask", bufs=1) as mk,
            tc.tile_pool(name="stage", bufs=3) as stage,
            tc.tile_pool(name="rrep", bufs=7) as rrp,
            tc.tile_pool(name="xm", bufs=5) as xmp,
            tc.tile_pool(name="outp", bufs=3) as outp,
            tc.tile_pool(name="psum", bufs=2, space=bass.MemorySpace.PSUM) as psp,
        ):
            wp = big.tile([128, 15 * 64], BF16)
            nc.sync.dma_start(wp[:], wp_d[:])

            # ---- depth -> r encoding (128h x 9*128w) ----
            dsh = mk.tile([128, 3 * 130], F32)
            nc.vector.memset(dsh[:], 0.0)
            nc.sync.dma_start(dsh[:, 131:259], dep_d[:, :])
            nc.sync.dma_start(dsh[0:127, 261:389], dep_d[1:128, :])
            nc.sync.dma_start(dsh[1:128, 1:129], dep_d[0:127, :])
            rfx = mk.tile([128, 1], F32)
            nc.sync.dma_start(rfx[:], rfx_d[:])

            g = mk.tile([128, 128], F32)
            nc.vector.tensor_scalar(g[:], dsh[:, 131:259], rfx[:], None, ALU.mult)
            rg = mk.tile([128, 128], F32)
            nc.vector.reciprocal(rg[:], g[:])

            def _win(base, offset, dims):
                return bass.AP(
                    base.tensor, offset, [list(base.ap[0])] + [list(d) for d in dims]
                )

            dcol = _win(dsh[:], 0, [(130, 3), (1, 3), (1, 128)])
            cent = _win(dsh[:], 131, [(0, 3), (0, 3), (1, 128)])
            rgb = _win(rg[:], 0, [(0, 9), (1, 128)])

            et = mk.tile([128, 9 * 128], F32)
            nc.vector.tensor_tensor(et[:], dcol, cent, ALU.subtract)
            tt = mk.tile([128, 9 * 128], F32)
            nc.vector.tensor_tensor(tt[:], et[:], rgb, ALU.mult)
            u1 = mk.tile([128, 9 * 128], F32)
            nc.vector.tensor_scalar(u1[:], tt[:], -1.5, None, ALU.is_ge)
            u2 = mk.tile([128, 9 * 128], F32)
            nc.vector.scalar_tensor_tensor(u2[:], tt[:], -0.5, u1[:], ALU.is_ge, ALU.add)
            u3 = mk.tile([128, 9 * 128], F32)
            nc.vector.scalar_tensor_tensor(u3[:], tt[:], 0.5, u2[:], ALU.is_ge, ALU.add)
            renc = mk.tile([128, 9 * 128], BF16)
            nc.vector.scalar_tensor_tensor(renc[:], tt[:], 1.5, u3[:], ALU.is_ge, ALU.add)

            # ---- rc: shuffle-source layout (bank*32+p rows) ----
            rc = big.tile([128, L], BF16)
            for p, (k1, k2) in enumerate(PATTERNS):
                for bank, k in ((0, k1), (1, k1), (2, k2), (3, k2)):
                    nc.gpsimd.dma_start(
                        rc[bank * 32 + p : bank * 32 + p + 1, :],
                        renc[:, k * 128 : (k + 1) * 128],
                    )

            # ---- x -> bf16, pair-shifted tiles ----
            tA = big.tile([128, XW], BF16)
            tB = big.tile([128, XW], BF16)
            nc.vector.memset(tA[:, 0:BASE], 0.0)
            nc.vector.memset(tA[:, BASE + L : XW], 0.0)
            nc.vector.memset(tB[:, 0:BASE], 0.0)
            nc.vector.memset(tB[:, BASE + L : XW], 0.0)
            for ci in range(NCHUNK):
                xs = stage.tile([C, CHUNK], F32)
                nc.sync.dma_start(xs[:], x_d[:, ci * CHUNK : (ci + 1) * CHUNK])
                nc.scalar.activation(
                    tA[0:64, BASE + ci * CHUNK : BASE + (ci + 1) * CHUNK], xs[:], AF.Copy
                )
            nc.gpsimd.dma_start(tA[64:128, BASE : BASE + L], tA[0:64, BASE + 1 : BASE + 1 + L])
            nc.gpsimd.dma_start(tB[0:64, BASE : BASE + L], tA[0:64, BASE : BASE + L])
            nc.gpsimd.dma_start(tB[64:128, BASE : BASE + L], tA[0:64, BASE + 128 : BASE + 128 + L])

            # ---- main loop ----
            for ci in range(NCHUNK):
                c0 = ci * CHUNK
                rreps = []
                for p in range(5):
                    rr = rrp.tile([128, CHUNK], BF16, tag="rr")
                    nc.vector.stream_shuffle(rr[:], rc[:, c0 : c0 + CHUNK], mask=[p] * 32)
                    rreps.append(rr)
                acc = psp.tile([O, CHUNK], F32)
                for b in range(3):
                    for p, (k1, k2) in enumerate(PATTERNS):
                        src = tB if p == 3 else tA
                        o = OFF[k1]
                        xm = xmp.tile([128, CHUNK], BF16, tag="xm")
                        nc.vector.scalar_tensor_tensor(
                            xm[:],
                            rreps[p][:],
                            float(3 - b),
                            src[:, BASE + c0 + o : BASE + c0 + o + CHUNK],
                            ALU.is_equal,
                            ALU.mult,
                        )
                        gidx = b * 5 + p
                        for h in range(CHUNK // 512):
                            nc.tensor.matmul(
                                acc[:, h * 512 : (h + 1) * 512],
                                wp[:, gidx * 64 : (gidx + 1) * 64],
                                xm[:, h * 512 : (h + 1) * 512],
                                start=(gidx == 0),
                                stop=(gidx == 14),
                            )
                osb = outp.tile([O, CHUNK], F32, tag="osb")
                nc.scalar.activation(osb[:], acc[:], AF.Copy)
                nc.sync.dma_start(out_d[:, c0 : c0 + CHUNK], osb[:])

    noop_cls = type(nc.sync.nop().ins)
    _split_excess_waits(nc, noop_cls, max_waits=1)
    return nc


def _prep_weights(w0, w1, w2):
    # lhsT per group g=b*5+p: rows 0-63 = W_b[:,:,k1].T, 64-127 = W_b[:,:,k2].T
    ws = [w0, w1, w2]
    wp = np.zeros((128, 15 * 64), dtype=np.float32)
    for b in range(3):
        wb = ws[b].reshape(O, C, 9)
        for p, (k1, k2) in enumerate(PATTERNS):
            g = b * 5 + p
            wp[0:64, g * 64 : (g + 1) * 64] = wb[:, :, k1].T
            if p != 4:  # pattern 4 second half stays zero (dup tap)
                wp[64:128, g * 64 : (g + 1) * 64] = wb[:, :, k2].T
    return wp.astype(ml_dtypes.bfloat16)


_CACHE = {}


def kernel(x, depth, fx, weight_0, weight_1, weight_2, _trace=False):
    x = np.asarray(x, dtype=np.float32)
    depth = np.asarray(depth, dtype=np.float32)
    fx = np.asarray(fx, dtype=np.float32)
    wp = _prep_weights(
        np.asarray(weight_0, np.float32),
        np.asarray(weight_1, np.float32),
        np.asarray(weight_2, np.float32),
    )
    in_maps = []
    for i in range(N_IMG):
        in_maps.append(
            {
                "x": np.ascontiguousarray(x[i].reshape(C, L)),
                "depth": np.ascontiguousarray(depth[i, 0]),
                "rfx": np.full((128, 1), 1.0 / fx[i], dtype=np.float32),
                "wp": wp,
            }
        )
    nc = _build_graph()
    res = run_bass_kernel_spmd(nc, in_maps, core_ids=list(range(N_IMG)), trace=_trace)
    out = np.stack([res.results[i]["out"].reshape(O, H, W) for i in range(N_IMG)])
    if _trace:
        return out.astype(np.float32), res
    return out.astype(np.float32)


if __name__ == "__main__":
    rng = np.random.default_rng(0)
    ins = {
        "x": rng.standard_normal((N_IMG, C, H, W), dtype=np.float32),
        "depth": (1.0 + 9.0 * rng.random((N_IMG, 1, H, W))).astype(np.float32),
        "fx": (400.0 + 200.0 * rng.random(N_IMG)).astype(np.float32),
        "weight_0": rng.standard_normal((O, C, 3, 3), dtype=np.float32) * 0.04,
        "weight_1": rng.standard_normal((O, C, 3, 3), dtype=np.float32) * 0.04,
        "weight_2": rng.standard_normal((O, C, 3, 3), dtype=np.float32) * 0.04,
    }
    out = kernel(**ins)
    print("ran ok", out.shape, out.dtype)



# revision 2
# speedup vs baseline: 1.9652x; 1.9652x over previous
"""Depth-masked 3-branch 3x3 conv (Conv2.5D) on 8 TRN2 NeuronCores.

Data-parallel over N=8 images (1 image/core). Per core:
  - center tap is structural: t=0 => r=2 => only branch 1 active, unmasked
    (one K=64 GEMM group straight from the x tile, no mask work)
  - remaining 24 (branch, tap) combos pair into 12 K=128 GEMM groups; each
    pair (ktop, kbot) has off(kbot)-off(ktop) in {1, W} so both halves read
    one shifted x tile (tA = [x | x+1], tB = [x | x+W])
  - branch masks precomputed compactly as bf16 0/1 planes, then expanded to
    [128, CHUNK] per group via three engine paths chosen to balance load:
    merged 2-row broadcast DMAs (8/chunk), PE selector-matmul broadcasts
    (3/chunk, drained by ACT), one DVE stream_shuffle (1/chunk)
  - masked inputs via tensor_tensor mult bf16 (2x DVE mode): 9 on DVE,
    3 on Pool; 26 matmuls/chunk accumulate all groups into one PSUM tile
"""

import sys

sys.path.insert(0, "/opt/trn_rl_repo")

import numpy as np
import ml_dtypes

import concourse.bass as bass
import concourse.mybir as mybir
from concourse.bass_utils import run_bass_kernel_spmd
from concourse import tile
from concourse.vector_clock import VectorClock, ScopedClock

F32 = mybir.dt.float32
BF16 = mybir.dt.bfloat16
AF = mybir.ActivationFunctionType
ALU = mybir.AluOpType

N_IMG, C, O, H, W = 8, 64, 64, 128, 128
L = H * W
CHUNK = 1024
NCHUNK = L // CHUNK
BASE = 256
XW = BASE + L + BASE
OFF = [(k // 3 - 1) * W + (k % 3 - 1) for k in range(9)]

# 12 paired groups: (ktop, kbot, btop, bbot). off(kbot)-off(ktop) is 1 or W.
# Chain split: first 8 expand masks via DMA broadcast, next 3 via PE
# selector-matmul, last 1 via stream_shuffle. Multiply engine: tiles 0-2 on
# Pool, rest on DVE.
TILES = [
    (0, 1, 0, 0),  # D0  tA
    (0, 1, 1, 1),  # D1  tA
    (1, 2, 2, 0),  # D2  tA
    (6, 7, 0, 0),  # D3  tA
    (2, 5, 1, 1),  # D4  tB
    (2, 5, 2, 2),  # D5  tB
    (3, 6, 1, 1),  # D6  tB
    (3, 6, 2, 2),  # D7  tB
    (7, 8, 1, 1),  # P0  tA
    (7, 8, 2, 2),  # P1  tA
    (0, 3, 2, 0),  # P2  tB
    (5, 8, 0, 0),  # S0  tB
]
N_DMA, N_PE = 8, 3
DMA_ROWS = [2, 4, 6, 8, 10, 12, 14, 16]  # rc partition of each D-tile's top row
PE_ROWS = [0, 32, 64]                    # rc partition pairs for P-tiles
SHUF_SLOT = 20                           # rc rows 20/52 = top, 84/116 = bottom
N_POOL_MULT = 3


def _patched_drain_and_barrier(self, tick_clock, wait_clock):
    # stock version puts every live sem wait on one drain -> walrus
    # "Too many sync wait commands"; emit one single-wait NOP per sem.
    ticks = list(tick_clock.global_clock)
    n = len(ticks)
    for i, t in enumerate(ticks):
        if t > 0:
            vec = [0] * n
            vec[i] = t
            nop = self.nc.sync.nop()
            wait_clock.add_sem_waits(nop.ins, ScopedClock({None: VectorClock(vec)}))
    self.nc.sync.drain()
    self.nc.all_engine_barrier()
    popped = self.nc._tile_sem_poison_stack.pop()
    assert popped is self._sem_poison
    self.nc.clear_and_free_semaphores(list(self.sems.allocated().values()))
    self.nc.all_engine_barrier()


tile.TileContext._drain_and_barrier = _patched_drain_and_barrier


def _split_excess_waits(nc, noop_cls, max_waits=1):
    # this walrus build rejects >1 sync-wait on several instruction
    # structs; hoist extras onto same-engine NoOps placed just before.
    for fn in nc.m.functions:
        for blk in fn.blocks:
            idx = 0
            while idx < len(blk.instructions):
                inst = blk.instructions[idx]
                si = inst.sync_info
                if si is not None and len(si.on_wait) > max_waits:
                    waits = list(si.on_wait)
                    si.on_wait = waits[-max_waits:]
                    pos = idx
                    for w in waits[:-max_waits]:
                        nop = noop_cls(
                            name=nc.get_next_instruction_name(), ins=[], outs=[]
                        )
                        nop.engine = inst.engine
                        nop.sync_info = mybir.SyncInfo(on_wait=[w], on_update=[])
                        nc.register_instruction(nop)
                        blk.instructions.insert(pos, nop)
                        pos += 1
                        idx += 1
                idx += 1


def _bcast2(ap):
    # [2, n] AP -> [2, 64, n] with a stride-0 middle dim: DMA source that
    # replicates each of the two rows across a 64-partition half.
    return bass.AP(
        ap.tensor,
        ap.offset,
        [list(ap.ap[0])] + [[0, 64]] + [list(d) for d in ap.ap[1:]],
    )


def _build_graph():
    nc = bass.Bass()
    x_d = nc.declare_dram_parameter("x", [C, L], F32, isOutput=False)
    dep_d = nc.declare_dram_parameter("depth", [H, W], F32, isOutput=False)
    rfx_d = nc.declare_dram_parameter("rfx", [128, 1], F32, isOutput=False)
    wp_d = nc.declare_dram_parameter("wp", [128, 13 * 64], BF16, isOutput=False)
    sel_d = nc.declare_dram_parameter("sel", [128, 128], BF16, isOutput=False)
    out_d = nc.declare_dram_parameter("out", [O, L], F32, isOutput=True)

    with tile.TileContext(nc) as tc:
        with (
            tc.tile_pool(name="big", bufs=1) as big,
            tc.tile_pool(name="mk", bufs=1) as mkg,
            tc.tile_pool(name="stage", bufs=2) as stage,
            tc.tile_pool(name="mask", bufs=10) as mskp,
            tc.tile_pool(name="xm", bufs=10) as xmp,
            tc.tile_pool(name="outp", bufs=3) as outp,
            tc.tile_pool(name="psum", bufs=2, space=bass.MemorySpace.PSUM) as psp,
        ):
            wp = big.tile([128, 13 * 64], BF16)
            nc.sync.dma_start(wp[:], wp_d[:])
            sel = big.tile([128, 128], BF16)
            nc.sync.dma_start(sel[:], sel_d[:])

            # ---- depth -> r codes (128h x 9*128w), then branch-mask planes
            dsh = mkg.tile([128, 3 * 130], F32)
            nc.vector.memset(dsh[:], 0.0)
            nc.sync.dma_start(dsh[:, 131:259], dep_d[:, :])
            nc.sync.dma_start(dsh[0:127, 261:389], dep_d[1:128, :])
            nc.sync.dma_start(dsh[1:128, 1:129], dep_d[0:127, :])
            rfx = mkg.tile([128, 1], F32)
            nc.sync.dma_start(rfx[:], rfx_d[:])

            g = mkg.tile([128, 128], F32)
            nc.vector.tensor_scalar(g[:], dsh[:, 131:259], rfx[:], None, ALU.mult)
            rg = mkg.tile([128, 128], F32)
            nc.vector.reciprocal(rg[:], g[:])

            def _win(base, offset, dims):
                return bass.AP(
                    base.tensor, offset, [list(base.ap[0])] + [list(d) for d in dims]
                )

            dcol = _win(dsh[:], 0, [(130, 3), (1, 3), (1, 128)])
            cent = _win(dsh[:], 131, [(0, 3), (0, 3), (1, 128)])
            rgb = _win(rg[:], 0, [(0, 9), (1, 128)])

            sA = mkg.tile([128, 9 * 128], F32)
            sB = mkg.tile([128, 9 * 128], F32)
            sC = mkg.tile([128, 9 * 128], F32)
            nc.vector.tensor_tensor(sA[:], dcol, cent, ALU.subtract)       # et
            nc.vector.tensor_tensor(sB[:], sA[:], rgb, ALU.mult)           # t
            nc.vector.tensor_scalar(sA[:], sB[:], -1.5, None, ALU.is_ge)   # u1
            nc.vector.scalar_tensor_tensor(sC[:], sB[:], -0.5, sA[:], ALU.is_ge, ALU.add)
            nc.vector.scalar_tensor_tensor(sA[:], sB[:], 0.5, sC[:], ALU.is_ge, ALU.add)
            renc = mkg.tile([128, 9 * 128], BF16)
            nc.vector.scalar_tensor_tensor(renc[:], sB[:], 1.5, sA[:], ALU.is_ge, ALU.add)

            mval = []
            for b in range(3):
                mv = mkg.tile([128, 9 * 128], BF16, tag=f"mval{b}", name=f"mval{b}")
                nc.vector.tensor_scalar(mv[:], renc[:], float(3 - b), None, ALU.is_equal)
                mval.append(mv)

            # ---- rc: broadcast-source rows [1, L] per (branch, tap) ----
            rc = big.tile([128, L], BF16)

            def fill_row(p, b, k):
                nc.gpsimd.dma_start(rc[p : p + 1, :], mval[b][:, k * 128 : (k + 1) * 128])

            for i in range(N_DMA):
                kt, kb, bt, bb = TILES[i]
                fill_row(DMA_ROWS[i], bt, kt)
                fill_row(DMA_ROWS[i] + 1, bb, kb)
            for j in range(N_PE):
                kt, kb, bt, bb = TILES[N_DMA + j]
                fill_row(PE_ROWS[j], bt, kt)
                fill_row(PE_ROWS[j] + 1, bb, kb)
            kt, kb, bt, bb = TILES[11]
            fill_row(SHUF_SLOT, bt, kt)
            fill_row(32 + SHUF_SLOT, bt, kt)
            fill_row(64 + SHUF_SLOT, bb, kb)
            fill_row(96 + SHUF_SLOT, bb, kb)

            # ---- x -> bf16 tiles tA = [x | x+1], tB = [x | x+W] ----
            tA = big.tile([128, XW], BF16)
            tB = big.tile([128, XW], BF16)
            nc.vector.memset(tA[:, 0:BASE], 0.0)
            nc.vector.memset(tA[:, BASE + L : XW], 0.0)
            nc.vector.memset(tB[:, 0:BASE], 0.0)
            nc.vector.memset(tB[:, BASE + L : XW], 0.0)
            SL = 1024
            for ci in range(L // SL):
                xs = stage.tile([C, SL], F32, tag="xs")
                nc.sync.dma_start(xs[:], x_d[:, ci * SL : (ci + 1) * SL])
                nc.scalar.activation(
                    tA[0:64, BASE + ci * SL : BASE + (ci + 1) * SL], xs[:], AF.Copy
                )
                nc.scalar.activation(
                    tB[0:64, BASE + ci * SL : BASE + (ci + 1) * SL], xs[:], AF.Copy
                )
            QL = L // 4
            for q in range(4):
                q0 = BASE + q * QL
                nc.sync.dma_start(
                    tA[64:128, q0 : q0 + QL], tA[0:64, q0 + 1 : q0 + 1 + QL]
                )
                nc.sync.dma_start(
                    tB[64:128, q0 : q0 + QL], tA[0:64, q0 + W : q0 + W + QL]
                )

            # ---- main loop ----
            for ci in range(NCHUNK):
                c0 = ci * CHUNK
                masks = [None] * 12
                # PE selector-matmul broadcasts -> PSUM f32 -> ACT -> bf16
                for j in range(N_PE):
                    t = N_DMA + j
                    pp = PE_ROWS[j]
                    mb = psp.tile([128, CHUNK], F32, tag="mbps", name="mbps")
                    for h in range(CHUNK // 512):
                        nc.tensor.matmul(
                            mb[:, h * 512 : (h + 1) * 512],
                            sel[pp : pp + 2, :],
                            rc[pp : pp + 2, c0 + h * 512 : c0 + (h + 1) * 512],
                            start=True,
                            stop=True,
                        )
                    mk = mskp.tile([128, CHUNK], BF16, tag="mk", name="mk")
                    nc.scalar.activation(mk[:], mb[:], AF.Copy)
                    masks[t] = mk
                # merged 2-row broadcast DMAs
                for i in range(N_DMA):
                    mk = mskp.tile([128, CHUNK], BF16, tag="mk", name="mkd")
                    nc.sync.dma_start(
                        mk[:], _bcast2(rc[DMA_ROWS[i] : DMA_ROWS[i] + 2, c0 : c0 + CHUNK])
                    )
                    masks[i] = mk
                # stream_shuffle broadcast
                mk = mskp.tile([128, CHUNK], BF16, tag="mk", name="mks")
                nc.vector.stream_shuffle(mk[:], rc[:, c0 : c0 + CHUNK], mask=[SHUF_SLOT] * 32)
                masks[11] = mk

                # masked inputs
                xms = [None] * 12
                for t in range(12):
                    kt, kb, bt, bb = TILES[t]
                    src = tA if OFF[kb] - OFF[kt] == 1 else tB
                    o = BASE + c0 + OFF[kt]
                    xm = xmp.tile([128, CHUNK], BF16, tag="xm", name="xm")
                    eng = nc.gpsimd if t < N_POOL_MULT else nc.vector
                    eng.tensor_tensor(xm[:], src[:, o : o + CHUNK], masks[t][:], ALU.mult)
                    xms[t] = xm

                # matmuls: group 0 = unmasked center (K=64), groups 1-12 = tiles
                acc = psp.tile([O, CHUNK], F32, tag="acc", name="acc")
                for h in range(CHUNK // 512):
                    nc.tensor.matmul(
                        acc[:, h * 512 : (h + 1) * 512],
                        wp[0:64, 0:64],
                        tA[0:64, BASE + c0 + h * 512 : BASE + c0 + (h + 1) * 512],
                        start=True,
                        stop=False,
                    )
                for t in range(12):
                    gidx = t + 1
                    for h in range(CHUNK // 512):
                        nc.tensor.matmul(
                            acc[:, h * 512 : (h + 1) * 512],
                            wp[:, gidx * 64 : (gidx + 1) * 64],
                            xms[t][:, h * 512 : (h + 1) * 512],
                            start=False,
                            stop=(gidx == 12),
                        )
                osb = outp.tile([O, CHUNK], F32, tag="osb")
                nc.scalar.activation(osb[:], acc[:], AF.Copy)
                nc.sync.dma_start(out_d[:, c0 : c0 + CHUNK], osb[:])

    noop_cls = type(nc.sync.nop().ins)
    _split_excess_waits(nc, noop_cls, max_waits=1)
    return nc


def _prep_weights(w0, w1, w2):
    ws = [w0.reshape(O, C, 9), w1.reshape(O, C, 9), w2.reshape(O, C, 9)]
    wp = np.zeros((128, 13 * 64), dtype=np.float32)
    wp[0:64, 0:64] = ws[1][:, :, 4].T  # center group
    for t, (kt, kb, bt, bb) in enumerate(TILES):
        g = t + 1
        wp[0:64, g * 64 : (g + 1) * 64] = ws[bt][:, :, kt].T
        wp[64:128, g * 64 : (g + 1) * 64] = ws[bb][:, :, kb].T
    return wp.astype(ml_dtypes.bfloat16)


def _prep_sel():
    sel = np.zeros((128, 128), dtype=ml_dtypes.bfloat16)
    for pp in PE_ROWS:
        sel[pp, 0:64] = 1.0
        sel[pp + 1, 64:128] = 1.0
    return sel


_CACHE = {}


def kernel(x, depth, fx, weight_0, weight_1, weight_2, _trace=False):
    x = np.asarray(x, dtype=np.float32)
    depth = np.asarray(depth, dtype=np.float32)
    fx = np.asarray(fx, dtype=np.float32)
    wp = _prep_weights(
        np.asarray(weight_0, np.float32),
        np.asarray(weight_1, np.float32),
        np.asarray(weight_2, np.float32),
    )
    sel = _prep_sel()
    in_maps = []
    for i in range(N_IMG):
        in_maps.append(
            {
                "x": np.ascontiguousarray(x[i].reshape(C, L)),
                "depth": np.ascontiguousarray(depth[i, 0]),
                "rfx": np.full((128, 1), 1.0 / fx[i], dtype=np.float32),
                "wp": wp,
                "sel": sel,
            }
        )
    nc = _build_graph()
    res = run_bass_kernel_spmd(nc, in_maps, core_ids=list(range(N_IMG)), trace=_trace)
    out = np.stack([res.results[i]["out"].reshape(O, H, W) for i in range(N_IMG)])
    if _trace:
        return out.astype(np.float32), res
    return out.astype(np.float32)


if __name__ == "__main__":
    rng = np.random.default_rng(0)
    ins = {
        "x": rng.standard_normal((N_IMG, C, H, W), dtype=np.float32),
        "depth": (1.0 + 9.0 * rng.random((N_IMG, 1, H, W))).astype(np.float32),
        "fx": (400.0 + 200.0 * rng.random(N_IMG)).astype(np.float32),
        "weight_0": rng.standard_normal((O, C, 3, 3), dtype=np.float32) * 0.04,
        "weight_1": rng.standard_normal((O, C, 3, 3), dtype=np.float32) * 0.04,
        "weight_2": rng.standard_normal((O, C, 3, 3), dtype=np.float32) * 0.04,
    }
    out = kernel(**ins)
    print("ran ok", out.shape, out.dtype)
